# revision 17
# baseline (speedup 1.0000x reference)
"""Trainium2 Bass kernel: batched QP projection (Dykstra fixed point) via an
active-set direct solve. Data parallel: 8 NeuronCores x 16 items each.

Per item (validated offline vs reference, rel err ~1.8e-3 with bf16 A):
  AAt = A A^T + eps I;  Mt ~= inv(AAt) (bf16 Newton-Schulz; preconditioner only)
  z0 = x - A^T h0 with AAt h0 = (A x - b)   (preconditioned Chebyshev)
  4 rounds: sigma = (z<0)&mask
     S = AAt - A_sig A_sig^T  ( = A D A^T + eps I, D = diag(1-sigma) )
     solve S w = t2 - A (D z0)  by Chebyshev (Mt-preconditioned), warm start
     z = z0 + A^T w
  out = x* - A^T h with AAt h = (A x* - b), x* = (1-sigma) z

A ships ONCE, bf16, in native [I*m, n] layout (one host cast, no transpose
passes); the device builds the A^T layout via PE-array transposes and streams
the native layout for A^T-matvecs. AAt is split hi/lo bf16 on device for
fp32-quality Chebyshev operators; vectors are split hi/lo bf16 where needed.
The jitted PJRT callable is cached so warm calls skip retrace/recompile.
"""

import sys

for _p in ("/opt/trn_rl_repo", "/opt/pypackages"):
    if _p not in sys.path:
        sys.path.insert(0, _p)

import numpy as np
import ml_dtypes
from contextlib import ExitStack

import concourse.bass as bass
import concourse.tile as tile
from concourse import mybir, bacc
from concourse.alu_op_type import AluOpType
from concourse.masks import make_identity

F32 = mybir.dt.float32
BF16 = mybir.dt.bfloat16

B, m, n = 128, 256, 1024
NCORES = 8
I = B // NCORES      # 16
KT = n // 128        # 8
MT = m // 128        # 2
IM = I * m           # 4096
IN = I * n           # 16384
EPS = 1e-6

N_ROUNDS = 3
RICH = [5, 4, 4]
NS_ITERS = 3
BND = 3            # boundary cheb iterations (z0/final solves)
AIN, BIN = 0.8340, 0.2173

_CACHE = {}


def _cheb_coeffs(l, u, iters):
    th, dl = (u + l) / 2.0, (u - l) / 2.0
    sg = th / dl
    out = []
    rho_prev = None
    for k in range(iters):
        if k == 0:
            out.append((0.0, 1.0 / th))
            rho_prev = 1.0 / sg
        else:
            rho = 1.0 / (2.0 * sg - rho_prev)
            out.append((rho * rho_prev, 2.0 * rho / dl))
            rho_prev = rho
    return out  # (beta_k, gamma_k): w_new = w + beta*(w - wprev) + gamma*z


def _build():
    nc = bacc.Bacc("TRN2", target_bir_lowering=False, debug=False, num_devices=NCORES)
    a_d = nc.declare_dram_parameter("a", [I * m, n], BF16, isOutput=False)
    xz_d = nc.declare_dram_parameter("xz", [128, KT * I], F32, isOutput=False)
    bc_d = nc.declare_dram_parameter("bc", [128, MT * I], F32, isOutput=False)
    m01_d = nc.declare_dram_parameter("m01", [128, KT * I], F32, isOutput=False)
    out_d = nc.declare_dram_parameter("out", [I, n], F32, isOutput=True)

    with tile.TileContext(nc) as tc, ExitStack() as ctx:
        nc = tc.nc
        ath_p = ctx.enter_context(tc.tile_pool(name="ath", bufs=1))
        res_p = ctx.enter_context(tc.tile_pool(name="res", bufs=1))
        scr_p = ctx.enter_context(tc.tile_pool(name="scr", bufs=2))
        msk_p = ctx.enter_context(tc.tile_pool(name="msk", bufs=4))
        str_p = ctx.enter_context(tc.tile_pool(name="str", bufs=3))
        vec_p = ctx.enter_context(tc.tile_pool(name="vec", bufs=1))
        ps_p = ctx.enter_context(tc.tile_pool(name="ps", bufs=2, space=bass.MemorySpace.PSUM))

        AT = [ath_p.tile([128, IM], BF16, name=f"ath{k}", tag=f"ath{k}") for k in range(KT)]
        AAth = [res_p.tile([128, IM], BF16, name=f"aah{k}", tag=f"aah{k}") for k in range(MT)]
        AAtl = [res_p.tile([128, IM], BF16, name=f"aal{k}", tag=f"aal{k}") for k in range(MT)]
        Mh = [res_p.tile([128, IM], BF16, name=f"mh{k}", tag=f"mh{k}") for k in range(MT)]
        IDL = [res_p.tile([128, IM], BF16, name=f"sh{k}", tag=f"sh{k}") for k in range(MT)]  # -> Sh later
        Sl = [res_p.tile([128, IM], BF16, name=f"sl{k}", tag=f"sl{k}") for k in range(MT)]
        Hb = [str_p.tile([128, IM], BF16, name=f"hbc{k}", tag="hbc", bufs=2) for k in range(MT)]  # NS-only

        zv = vec_p.tile([128, KT * I], F32, name="zv", tag="zv")
        z0v = vec_p.tile([128, KT * I], F32, name="z0v", tag="z0v")
        uv = vec_p.tile([128, KT * I], F32, name="uv", tag="uv")
        sig = vec_p.tile([128, KT * I], F32, name="sig", tag="sig")
        m01v = vec_p.tile([128, KT * I], F32, name="m01v", tag="m01v")
        xzv = vec_p.tile([128, KT * I], F32, name="xzv", tag="xzv")
        ubi = vec_p.tile([128, 2 * KT * I], BF16, name="ubi", tag="ubi")
        ztmp = vec_p.tile([128, KT * I], F32, name="ztmp", tag="ztmp")
        bcol = vec_p.tile([128, MT * I], F32, name="bcol", tag="bcol")
        gcol = vec_p.tile([128, MT * I], F32, name="gcol", tag="gcol")
        hcol = vec_p.tile([128, MT * I], F32, name="hcol", tag="hcol")
        wcol = vec_p.tile([128, MT * I], F32, name="wcol", tag="wcol")
        wprev = vec_p.tile([128, MT * I], F32, name="wprev", tag="wprev")
        wtmp = vec_p.tile([128, MT * I], F32, name="wtmp", tag="wtmp")
        t2col = vec_p.tile([128, MT * I], F32, name="t2col", tag="t2col")
        rhsc = vec_p.tile([128, MT * I], F32, name="rhsc", tag="rhsc")
        rcol = vec_p.tile([128, MT * I], F32, name="rcol", tag="rcol")
        mtmp = vec_p.tile([128, MT * I], F32, name="mtmp", tag="mtmp")
        gbh = vec_p.tile([128, MT * I], BF16, name="gbh", tag="gbh")
        gbi = vec_p.tile([128, 2 * MT * I], BF16, name="gbi", tag="gbi")
        idt = vec_p.tile([128, 128], BF16, name="idt", tag="idt")

        # ---------------- helpers ----------------
        def split_i(dst, src, tmp):
            """dst bf16 [128, 2*C] interleaved (hi,lo) pairs from f32 src."""
            d3 = dst[:].rearrange("p (c t) -> p c t", t=2)
            nc.vector.tensor_copy(d3[:, :, 0], src[:])
            nc.vector.tensor_tensor(tmp[:], src[:], d3[:, :, 0], AluOpType.subtract)
            nc.vector.tensor_copy(d3[:, :, 1], tmp[:])

        def mm_batch(passes, kts, post):
            """out[i] = sum_passes lhsT[i].T @ rhs[i] over kts; psum chunks of
            8 items; post(mt, g0, GI, ps)."""
            GI = 8
            for mt in range(MT):
                for g0 in range(0, I, GI):
                    ps = ps_p.tile([128, 2048], F32, name="psb", tag="psb")
                    npass = len(passes)
                    for ki, kt in enumerate(kts):
                        for pi, (lhs_t, rhs_t) in enumerate(passes):
                            for gi in range(GI):
                                i = g0 + gi
                                nc.tensor.matmul(
                                    ps[:, gi * m:(gi + 1) * m],
                                    lhs_t[kt][:, i * m + mt * 128: i * m + mt * 128 + 128],
                                    rhs_t[kt][:, i * m:(i + 1) * m],
                                    start=(pi == 0 and ki == 0 and gi % 2 == 0),
                                    stop=(pi == npass - 1 and ki == len(kts) - 1
                                          and gi % 2 == 1),
                                )
                    post(mt, g0, GI, ps)

        def s_build(last):
            """S = AAt - A_sig A_sig^T; Sh (+Sl if last). Mask lhs once per
            (kt,item); both mt psums live."""
            GI = 8
            for g0 in range(0, I, GI):
                pss = [ps_p.tile([128, 2048], F32, name="psb", tag="psb") for _ in range(MT)]
                for ki, kt in enumerate(range(KT)):
                    for gi in range(GI):
                        i = g0 + gi
                        mk_hi = msk_p.tile([128, m], BF16, name="mskh", tag="mskh")
                        nc.vector.tensor_scalar(
                            mk_hi[:], AT[kt][:, i * m:(i + 1) * m],
                            sig[:, kt * I + i:kt * I + i + 1], None, AluOpType.mult)
                        for mt in range(MT):
                            nc.tensor.matmul(
                                pss[mt][:, gi * m:(gi + 1) * m],
                                mk_hi[:, mt * 128:mt * 128 + 128],
                                AT[kt][:, i * m:(i + 1) * m],
                                start=(ki == 0 and gi % 2 == 0),
                                stop=(ki == KT - 1 and gi % 2 == 1))
                for mt in range(MT):
                    sl_c = slice(g0 * m, (g0 + GI) * m)
                    tmp = scr_p.tile([128, 2048], F32, name="chunk", tag="chunk")
                    nc.vector.tensor_copy(tmp[:], AAtl[mt][:, sl_c])
                    nc.vector.tensor_tensor(tmp[:], tmp[:], pss[mt][:], AluOpType.subtract)
                    nc.vector.tensor_tensor(tmp[:], AAth[mt][:, sl_c], tmp[:], AluOpType.add)
                    nc.vector.tensor_copy(Sh[mt][:, sl_c], tmp[:])
                    if last:
                        nc.vector.tensor_tensor(tmp[:], tmp[:], Sh[mt][:, sl_c],
                                                AluOpType.subtract)
                        nc.vector.tensor_copy(Sl[mt][:, sl_c], tmp[:])

        a_rows = a_d.rearrange("(i p) n -> i p n", p=m)  # [I, m, n]

        def l1hi_stream(ki, c0, CH):
            t = str_p.tile([128, 2048], BF16, name="l1c", tag="l1c", bufs=2)
            src = a_rows[c0:c0 + CH, ki * 128:(ki + 1) * 128, :].rearrange(
                "i p n -> p i n")
            nc.gpsimd.dma_start(out=t[:].rearrange("p (i n) -> p i n", n=n),
                                in_=src)
            return t[:]

        def msp_mv(col_out, vi, Qh, Ql, split):
            """col_out[:, o*I+i] = sum_k Q[o,k](i) @ v[k](i); vi interleaved
            (hi,lo) bf16 pairs; Q symmetric, SBUF-resident."""
            pt = ps_p.tile([128, 2048], F32, name="psb", tag="psb")
            ops = [Qh, Ql] if split else [Qh]
            for o in range(MT):
                for i in range(I):
                    c2 = 2 * (o * I + i)
                    for pi, Q in enumerate(ops):
                        for k in range(MT):
                            nc.tensor.matmul(
                                pt[:, c2:c2 + 2],
                                Q[k][:, i * m + o * 128: i * m + o * 128 + 128],
                                vi[:, 2 * (k * I + i): 2 * (k * I + i) + 2],
                                start=(pi == 0 and k == 0),
                                stop=(pi == len(ops) - 1 and k == MT - 1))
            pe = pt[:, 0:2 * MT * I].rearrange("p (c t) -> p c t", t=2)
            nc.vector.tensor_copy(mtmp[:], pe[:, :, 0])
            nc.vector.tensor_tensor(col_out[:], mtmp[:], pe[:, :, 1],
                                    AluOpType.add)

        def prec_mv(col_out, vh):
            pt = ps_p.tile([128, 2048], F32, name="psb", tag="psb")
            for o in range(MT):
                for i in range(I):
                    c = o * I + i
                    for k in range(MT):
                        nc.tensor.matmul(
                            pt[:, c:c + 1],
                            Mh[k][:, i * m + o * 128: i * m + o * 128 + 128],
                            vh[:, k * I + i: k * I + i + 1],
                            start=(k == 0), stop=(k == MT - 1))
            nc.vector.tensor_copy(col_out[:], pt[:, 0:MT * I])

        def dn_mv(col_out, ui):
            """A @ v (n -> m), ui interleaved n-space pairs."""
            pt = ps_p.tile([128, 2048], F32, name="psb", tag="psb")
            for o in range(MT):
                for i in range(I):
                    c2 = 2 * (o * I + i)
                    for kt in range(KT):
                        nc.tensor.matmul(
                            pt[:, c2:c2 + 2],
                            AT[kt][:, i * m + o * 128: i * m + o * 128 + 128],
                            ui[:, 2 * (kt * I + i): 2 * (kt * I + i) + 2],
                            start=(kt == 0), stop=(kt == KT - 1))
            pe = pt[:, 0:2 * MT * I].rearrange("p (c t) -> p c t", t=2)
            nc.vector.tensor_copy(mtmp[:], pe[:, :, 0])
            nc.vector.tensor_tensor(col_out[:], mtmp[:], pe[:, :, 1],
                                    AluOpType.add)

        def up_mv(col_out, gi_t):
            """A^T @ w (m -> n), gi_t interleaved m-space pairs; A row-layout
            blocks streamed from DRAM as stationaries."""
            ov = col_out.rearrange("p (t i) -> p t i", i=I)
            for c0 in range(0, I, 2):
                tiles = [l1hi_stream(mt, c0, 2) for mt in range(MT)]
                pt = ps_p.tile([128, 2048], F32, name="psb", tag="psb")
                for i_rel in range(2):
                    i = c0 + i_rel
                    for kt in range(KT):
                        c2 = 2 * (i_rel * KT + kt)
                        for mt in range(MT):
                            nc.tensor.matmul(
                                pt[:, c2:c2 + 2],
                                tiles[mt][:, i_rel * n + kt * 128:
                                          i_rel * n + (kt + 1) * 128],
                                gi_t[:, 2 * (mt * I + i): 2 * (mt * I + i) + 2],
                                start=(mt == 0), stop=(mt == MT - 1))
                pe = pt[:, 0:4 * KT].rearrange("p (i t w) -> p t i w", i=2, w=2)
                me = mtmp[:, 0:2 * KT].rearrange("p (t i) -> p t i", i=2)
                nc.vector.tensor_copy(me, pe[:, :, :, 0])
                nc.vector.tensor_tensor(ov[:, :, c0:c0 + 2],
                                        me, pe[:, :, :, 1],
                                        AluOpType.add)

        def cheb(Qh, Ql, iters, l, u, use_split, warm):
            for k, (beta, gamma) in enumerate(_cheb_coeffs(l, u, iters)):
                if k == 0 and not warm:
                    # w=0: residual is exactly rhs; skip the operator apply
                    nc.vector.tensor_copy(gbh[:], rhsc[:])
                    prec_mv(mtmp, gbh)
                    nc.vector.tensor_scalar(wcol[:], mtmp[:], gamma, None, AluOpType.mult)
                    nc.vector.tensor_copy(wprev[:], wcol[:])
                    continue
                split_i(gbi, wcol, mtmp)
                msp_mv(rcol, gbi, Qh, Ql, use_split)
                nc.vector.tensor_tensor(rcol[:], rhsc[:], rcol[:], AluOpType.subtract)
                nc.vector.tensor_copy(gbh[:], rcol[:])
                prec_mv(mtmp, gbh)
                nc.vector.tensor_tensor(wtmp[:], wcol[:], wprev[:], AluOpType.subtract)
                nc.vector.tensor_copy(wprev[:], wcol[:])
                nc.vector.scalar_tensor_tensor(wtmp[:], wtmp[:], beta, wcol[:],
                                               AluOpType.mult, AluOpType.add)
                nc.vector.scalar_tensor_tensor(wcol[:], mtmp[:], gamma, wtmp[:],
                                               AluOpType.mult, AluOpType.add)

        # ============ loads ============
        nc.sync.dma_start(out=xzv[:], in_=xz_d[:])
        nc.sync.dma_start(out=bcol[:], in_=bc_d[:])
        nc.sync.dma_start(out=m01v[:], in_=m01_d[:])
        make_identity(nc, idt[:])
        # IDL[mt][p, i*m + mt*128 + p] = 1 (identity blocks, all items)
        for mt in range(MT):
            nc.gpsimd.memset(IDL[mt][:], 0.0)
            for i in range(I):
                nc.gpsimd.affine_select(
                    out=IDL[mt][:, i * m:(i + 1) * m],
                    in_=IDL[mt][:, i * m:(i + 1) * m],
                    compare_op=AluOpType.not_equal,
                    fill=1.0,
                    base=mt * 128,
                    pattern=[[-1, m]],
                    channel_multiplier=1,
                )
        # A native rows -> AT (A^T layout) via PE transposes
        for mt in range(MT):
            for i0 in range(0, I, 2):
                ps_t = ps_p.tile([128, 2048], BF16, name="psb", tag="psb")
                for di in range(2):
                    i = i0 + di
                    ac = str_p.tile([128, 2048], BF16, name="l1c", tag="l1c", bufs=2)
                    nc.gpsimd.dma_start(out=ac[:, 0:n], in_=a_rows[i, mt * 128:(mt + 1) * 128, :])
                    for kt in range(KT):
                        nc.tensor.transpose(
                            ps_t[:, di * 1024 + kt * 128: di * 1024 + (kt + 1) * 128],
                            ac[:, kt * 128:(kt + 1) * 128], idt[:])
                for di in range(2):
                    i = i0 + di
                    for kt in range(KT):
                        nc.vector.tensor_copy(
                            AT[kt][:, i * m + mt * 128: i * m + mt * 128 + 128],
                            ps_t[:, di * 1024 + kt * 128: di * 1024 + (kt + 1) * 128])

        # ============ AAt = A A^T + eps I ============
        def post_aat(mt, g0, GI, ps):
            sl_c = slice(g0 * m, (g0 + GI) * m)
            tmp = scr_p.tile([128, 2048], F32, name="chunk", tag="chunk")
            nc.vector.scalar_tensor_tensor(tmp[:], IDL[mt][:, sl_c], EPS, ps[:],
                                           AluOpType.mult, AluOpType.add)
            nc.vector.tensor_copy(AAth[mt][:, sl_c], tmp[:])
            nc.vector.tensor_tensor(tmp[:], tmp[:], AAth[mt][:, sl_c], AluOpType.subtract)
            nc.vector.tensor_copy(AAtl[mt][:, sl_c], tmp[:])
        mm_batch([(AT, AT)], range(KT), post_aat)

        # ============ Mt: Newton-Schulz bf16 ============
        assert NS_ITERS % 2 == 1
        Xbufs = [Sl, Mh]   # ping-pong; X0 -> Sl, final (odd) lands in Mh
        for mt in range(MT):
            for c0 in range(0, IM, 2048):
                tmp = scr_p.tile([128, 2048], F32, name="chunk", tag="chunk")
                nc.vector.tensor_scalar(tmp[:], AAth[mt][:, c0:c0 + 2048], -BIN, None,
                                        AluOpType.mult)
                nc.vector.scalar_tensor_tensor(tmp[:], IDL[mt][:, c0:c0 + 2048], AIN,
                                               tmp[:], AluOpType.mult, AluOpType.add)
                nc.vector.tensor_copy(Xbufs[0][mt][:, c0:c0 + 2048], tmp[:])
        for it in range(NS_ITERS):
            Xcur = Xbufs[it % 2]
            Xnxt = Xbufs[(it + 1) % 2]
            def post_p1(mt, g0, GI, ps):
                nc.vector.tensor_copy(Hb[mt][:, g0 * m:(g0 + GI) * m], ps[:])
            mm_batch([(AAth, Xcur)], range(MT), post_p1)
            def post_p2(mt, g0, GI, ps, Xc=Xcur, Xn=Xnxt):
                sl_c = slice(g0 * m, (g0 + GI) * m)
                nc.vector.scalar_tensor_tensor(Xn[mt][:, sl_c], Xc[mt][:, sl_c], 2.0,
                                               ps[:], AluOpType.mult, AluOpType.subtract)
            mm_batch([(Xcur, Hb)], range(MT), post_p2)

        # ============ z0, t2 ============
        split_i(ubi, xzv, ztmp)
        dn_mv(gcol, ubi)
        nc.vector.tensor_tensor(gcol[:], gcol[:], bcol[:], AluOpType.subtract)
        nc.vector.tensor_copy(rhsc[:], gcol[:])
        cheb(AAth, AAtl, BND, 0.80, 1.25, True, warm=False)
        nc.vector.tensor_copy(hcol[:], wcol[:])
        # t2 = b + eps*(AAt^-1 b): the eps term is ~1e-6*|b|, below the bf16-A
        # error floor by 4 orders; use t2 = b directly.
        nc.vector.tensor_copy(t2col[:], bcol[:])
        split_i(gbi, hcol, mtmp)
        up_mv(z0v, gbi)
        nc.vector.tensor_tensor(z0v[:], xzv[:], z0v[:], AluOpType.subtract)

        # ============ rounds ============
        nc.vector.tensor_copy(zv[:], z0v[:])
        Sh = IDL  # identity dead from here; tags sh0/sh1 reused as Sh
        for r in range(N_ROUNDS):
            last = r == N_ROUNDS - 1
            nc.vector.tensor_scalar(sig[:], zv[:], 0.0, None, AluOpType.is_lt)
            nc.vector.tensor_tensor(sig[:], sig[:], m01v[:], AluOpType.mult)
            s_build(last)
            nc.vector.scalar_tensor_tensor(uv[:], sig[:], 0.0, z0v[:],
                                           AluOpType.is_equal, AluOpType.mult)
            split_i(ubi, uv, ztmp)
            dn_mv(rhsc, ubi)
            nc.vector.tensor_tensor(rhsc[:], t2col[:], rhsc[:], AluOpType.subtract)
            cheb(Sh, Sl, RICH[r], 0.07, 1.30, use_split=last, warm=(r > 0))
            split_i(gbi, wcol, mtmp)
            up_mv(zv, gbi)
            nc.vector.tensor_tensor(zv[:], z0v[:], zv[:], AluOpType.add)

        # ============ final ============
        nc.vector.tensor_scalar(sig[:], zv[:], 0.0, None, AluOpType.is_lt)
        nc.vector.tensor_tensor(sig[:], sig[:], m01v[:], AluOpType.mult)
        nc.vector.scalar_tensor_tensor(uv[:], sig[:], 0.0, zv[:],
                                       AluOpType.is_equal, AluOpType.mult)
        split_i(ubi, uv, ztmp)
        dn_mv(gcol, ubi)
        nc.vector.tensor_tensor(gcol[:], gcol[:], bcol[:], AluOpType.subtract)
        nc.vector.tensor_copy(rhsc[:], gcol[:])
        cheb(AAth, AAtl, BND, 0.80, 1.25, True, warm=False)
        split_i(gbi, wcol, mtmp)
        up_mv(ztmp, gbi)
        nc.vector.tensor_tensor(ztmp[:], uv[:], ztmp[:], AluOpType.subtract)
        for i in range(I):
            src = ztmp.rearrange("p (t i) -> p t i", i=I)[:, :, i]
            dst = out_d[i, :].rearrange("(t p) -> p t", p=128)
            nc.sync.dma_start(out=dst, in_=src)

    nc.compile()
    return nc


_SHIMMED = False


def _fix_cc_flags():
    """Route static DMAs through SP so multi-wait DMAs are legal walrus
    codegen (the embedded-wait form only fits one sync wait)."""
    global _SHIMMED
    try:
        from concourse.compiler_utils import get_compiler_flags, set_compiler_flags
        flags = get_compiler_flags()
        nf = [f.replace("--assign-static-dmas-to-sp=false",
                        "--assign-static-dmas-to-sp=true") for f in flags]
        if nf != flags:
            set_compiler_flags(nf)
    except Exception:
        pass
    if not _SHIMMED:
        import concourse.bass_utils as BU
        orig = BU.run_command

        def patched(cmd, *a, **k):
            if isinstance(cmd, (list, tuple)):
                cmd = [str(c).replace("--assign-static-dmas-to-sp=false",
                                      "--assign-static-dmas-to-sp=true") for c in cmd]
            return orig(cmd, *a, **k)

        BU.run_command = patched
        _SHIMMED = True


def _get_runner():
    """Build (once) the Bass module and a cached jitted shard_map callable.
    Returns run(arrays: list[np.ndarray]) -> np.ndarray [B, n]."""
    if "runner" in _CACHE:
        return _CACHE["runner"]
    _fix_cc_flags()
    import jax
    from jax.sharding import Mesh, PartitionSpec
    from jax.experimental.shard_map import shard_map
    from concourse.bass2jax import _bass_exec_p, partition_id_tensor, install_neuronx_cc_hook

    install_neuronx_cc_hook()
    nc = _build()

    partition_name = nc.partition_id_tensor.name if nc.partition_id_tensor else None
    in_names, out_names, out_avals, zero_shapes = [], [], [], []
    for alloc in nc.m.functions[0].allocations:
        if not isinstance(alloc, mybir.MemoryLocationSet):
            continue
        name = alloc.memorylocations[0].name
        if alloc.kind == "ExternalInput":
            if name != partition_name:
                in_names.append(name)
        elif alloc.kind == "ExternalOutput":
            shape = tuple(alloc.tensor_shape)
            dtype = mybir.dt.np(alloc.dtype)
            out_names.append(name)
            out_avals.append(jax.core.ShapedArray(shape, dtype))
            zero_shapes.append((shape, dtype))
    n_params = len(in_names)
    n_outs = len(out_avals)
    in_names_all = in_names + out_names + ([partition_name] if partition_name else [])

    def _body(*args):
        operands = list(args)
        if partition_name is not None:
            operands.append(partition_id_tensor())
        outs = _bass_exec_p.bind(
            *operands,
            out_avals=tuple(out_avals),
            in_names=tuple(in_names_all),
            out_names=tuple(out_names),
            lowering_input_output_aliases=(),
            sim_require_finite=True,
            sim_require_nnan=True,
            nc=nc,
        )
        return tuple(outs)

    devices = jax.devices()[:NCORES]
    mesh = Mesh(np.asarray(devices), ("core",))
    in_specs = (PartitionSpec("core"),) * (n_params + n_outs)
    out_specs = (PartitionSpec("core"),) * n_outs
    donate = tuple(range(n_params, n_params + n_outs))
    sharded = jax.jit(
        shard_map(_body, mesh=mesh, in_specs=in_specs, out_specs=out_specs,
                  check_rep=False),
        donate_argnums=donate, keep_unused=True)

    def run(arr_map):
        args = [arr_map[nm] for nm in in_names]
        zeros = [np.zeros((NCORES * s[0], *s[1:]), dt) for s, dt in zero_shapes]
        outs = sharded(*args, *zeros)
        out = np.asarray(outs[out_names.index("out")])
        return out.reshape(B, n)

    _CACHE["parts"] = dict(sharded=sharded, in_names=in_names,
                           out_names=out_names, zero_shapes=zero_shapes,
                           mesh=mesh, nc=nc)
    _CACHE["runner"] = run
    return run


def _prep_globals(x, b, A, mask):
    a_g = np.ascontiguousarray(A.reshape(B * m, n)).astype(ml_dtypes.bfloat16)
    xz_g = np.ascontiguousarray(
        x.reshape(NCORES, I, KT, 128).transpose(0, 3, 2, 1)).reshape(NCORES * 128, KT * I)
    bc_g = np.ascontiguousarray(
        b.reshape(NCORES, I, MT, 128).transpose(0, 3, 2, 1)).reshape(NCORES * 128, MT * I)
    m01_1 = np.ascontiguousarray(
        np.broadcast_to(mask.reshape(KT, 128, 1), (KT, 128, I)).transpose(1, 0, 2)
    ).reshape(128, KT * I).astype(np.float32)
    m01_g = np.ascontiguousarray(np.tile(m01_1, (NCORES, 1)))
    return {"a": a_g, "xz": xz_g.astype(np.float32), "bc": bc_g.astype(np.float32),
            "m01": m01_g}


def kernel(x, b, A, nonnegative_mask):
    x = np.asarray(x, dtype=np.float32)
    b = np.asarray(b, dtype=np.float32)
    A = np.asarray(A, dtype=np.float32)
    mk = np.asarray(nonnegative_mask).astype(np.float32)
    run = _get_runner()
    arr_map = _prep_globals(x, b, A, mk)
    return np.ascontiguousarray(run(arr_map)).astype(np.float32)


# revision 18
# speedup vs baseline: 1.4168x; 1.4168x over previous
"""Trainium2 Bass kernel: batched QP projection (Dykstra fixed point) via an
active-set direct solve. Data parallel: 8 NeuronCores x 16 items each.

Per item (validated offline vs reference, rel err ~2.0e-3, gate 2e-2; the
error floor is bf16 quantization of A, not solver convergence):
  AAt = A A^T + eps I;  Mt ~= inv(AAt) (bf16 Newton-Schulz; preconditioner)
  z0 = x - A^T h0 with AAt h0 = (A x - b)   (preconditioned Chebyshev)
  3 rounds: sigma = (z<0)&mask
     S = AAt - A_sig A_sig^T  ( = A D A^T + eps I, D = diag(1-sigma) )
     solve S w = b - A (D z0)  by Chebyshev (Mt-preconditioned), warm start
     z = z0 + A^T w
  out = x* - A^T h with AAt h = (A x* - b), x* = (1-sigma) z

A ships ONCE, bf16, in native [I*m, n] layout (one host cast, no transpose
passes, no mask permute); the device builds the A^T layout via PE-array
transposes and streams the native layout for A^T-matvecs. AAt/S are split
hi/lo bf16 on device for fp32-quality Chebyshev operators. All matvecs are
thin per-item matmuls (matrix stationary, interleaved hi/lo vector pairs as
2-wide rhs) writing results directly in column layout to PSUM - no DRAM
bounce, and the vector hi/lo split costs nothing extra. The jitted PJRT
callable is cached so warm calls skip retrace/recompile.
"""

import sys

for _p in ("/opt/trn_rl_repo", "/opt/pypackages"):
    if _p not in sys.path:
        sys.path.insert(0, _p)

import numpy as np
import ml_dtypes
from contextlib import ExitStack

import concourse.bass as bass
import concourse.tile as tile
from concourse import mybir, bacc
from concourse.alu_op_type import AluOpType
from concourse.masks import make_identity

F32 = mybir.dt.float32
BF16 = mybir.dt.bfloat16

B, m, n = 128, 256, 1024
NCORES = 8
I = B // NCORES      # 16
KT = n // 128        # 8
MT = m // 128        # 2
IM = I * m           # 4096
IN = I * n           # 16384
EPS = 1e-6

N_ROUNDS = 3
RICH = [5, 4, 4]
NS_ITERS = 3
BND = 3            # boundary cheb iterations (z0/final solves)
AIN, BIN = 0.8340, 0.2173

_CACHE = {}


def _cheb_coeffs(l, u, iters):
    th, dl = (u + l) / 2.0, (u - l) / 2.0
    sg = th / dl
    out = []
    rho_prev = None
    for k in range(iters):
        if k == 0:
            out.append((0.0, 1.0 / th))
            rho_prev = 1.0 / sg
        else:
            rho = 1.0 / (2.0 * sg - rho_prev)
            out.append((rho * rho_prev, 2.0 * rho / dl))
            rho_prev = rho
    return out  # (beta_k, gamma_k): w_new = w + beta*(w - wprev) + gamma*z


def _build():
    nc = bacc.Bacc("TRN2", target_bir_lowering=False, debug=False, num_devices=NCORES)
    a_d = nc.declare_dram_parameter("a", [I * m, n], BF16, isOutput=False)
    xz_d = nc.declare_dram_parameter("xz", [128, KT * I], F32, isOutput=False)
    bc_d = nc.declare_dram_parameter("bc", [128, MT * I], F32, isOutput=False)
    m01_d = nc.declare_dram_parameter("m01", [128, KT * I], F32, isOutput=False)
    out_d = nc.declare_dram_parameter("out", [I, n], F32, isOutput=True)

    with tile.TileContext(nc) as tc, ExitStack() as ctx:
        nc = tc.nc
        ath_p = ctx.enter_context(tc.tile_pool(name="ath", bufs=1))
        res_p = ctx.enter_context(tc.tile_pool(name="res", bufs=1))
        scr_p = ctx.enter_context(tc.tile_pool(name="scr", bufs=2))
        msk_p = ctx.enter_context(tc.tile_pool(name="msk", bufs=4))
        str_p = ctx.enter_context(tc.tile_pool(name="str", bufs=3))
        vec_p = ctx.enter_context(tc.tile_pool(name="vec", bufs=1))
        ps_p = ctx.enter_context(tc.tile_pool(name="ps", bufs=2, space=bass.MemorySpace.PSUM))

        AT = [ath_p.tile([128, IM], BF16, name=f"ath{k}", tag=f"ath{k}") for k in range(KT)]
        AAth = [res_p.tile([128, IM], BF16, name=f"aah{k}", tag=f"aah{k}") for k in range(MT)]
        AAtl = [res_p.tile([128, IM], BF16, name=f"aal{k}", tag=f"aal{k}") for k in range(MT)]
        Mh = [res_p.tile([128, IM], BF16, name=f"mh{k}", tag=f"mh{k}") for k in range(MT)]
        IDL = [res_p.tile([128, IM], BF16, name=f"sh{k}", tag=f"sh{k}") for k in range(MT)]  # -> Sh later
        Sl = [res_p.tile([128, IM], BF16, name=f"sl{k}", tag=f"sl{k}") for k in range(MT)]
        Hb = [str_p.tile([128, IM], BF16, name=f"hbc{k}", tag="hbc", bufs=2) for k in range(MT)]  # NS-only

        zv = vec_p.tile([128, KT * I], F32, name="zv", tag="zv")
        z0v = vec_p.tile([128, KT * I], F32, name="z0v", tag="z0v")
        uv = vec_p.tile([128, KT * I], F32, name="uv", tag="uv")
        sig = vec_p.tile([128, KT * I], F32, name="sig", tag="sig")
        m01v = vec_p.tile([128, KT * I], F32, name="m01v", tag="m01v")
        xzv = vec_p.tile([128, KT * I], F32, name="xzv", tag="xzv")
        ubi = vec_p.tile([128, 2 * KT * I], BF16, name="ubi", tag="ubi")
        ztmp = vec_p.tile([128, KT * I], F32, name="ztmp", tag="ztmp")
        bcol = vec_p.tile([128, MT * I], F32, name="bcol", tag="bcol")
        gcol = vec_p.tile([128, MT * I], F32, name="gcol", tag="gcol")
        hcol = vec_p.tile([128, MT * I], F32, name="hcol", tag="hcol")
        wcol = vec_p.tile([128, MT * I], F32, name="wcol", tag="wcol")
        wprev = vec_p.tile([128, MT * I], F32, name="wprev", tag="wprev")
        wtmp = vec_p.tile([128, MT * I], F32, name="wtmp", tag="wtmp")
        t2col = vec_p.tile([128, MT * I], F32, name="t2col", tag="t2col")
        rhsc = vec_p.tile([128, MT * I], F32, name="rhsc", tag="rhsc")
        rcol = vec_p.tile([128, MT * I], F32, name="rcol", tag="rcol")
        mtmp = vec_p.tile([128, MT * I], F32, name="mtmp", tag="mtmp")
        gbh = vec_p.tile([128, MT * I], BF16, name="gbh", tag="gbh")
        gbi = vec_p.tile([128, 2 * MT * I], BF16, name="gbi", tag="gbi")
        idt = vec_p.tile([128, 128], BF16, name="idt", tag="idt")

        # ---------------- helpers ----------------
        def split_i(dst, src, tmp):
            """dst bf16 [128, 2*C] interleaved (hi,lo) pairs from f32 src."""
            d3 = dst[:].rearrange("p (c t) -> p c t", t=2)
            nc.vector.tensor_copy(d3[:, :, 0], src[:])
            nc.vector.tensor_tensor(tmp[:], src[:], d3[:, :, 0], AluOpType.subtract)
            nc.vector.tensor_copy(d3[:, :, 1], tmp[:])

        def mm_batch(passes, kts, post):
            """out[i] = sum_passes lhsT[i].T @ rhs[i] over kts; psum chunks of
            8 items; post(mt, g0, GI, ps)."""
            GI = 8
            for mt in range(MT):
                for g0 in range(0, I, GI):
                    ps = ps_p.tile([128, 2048], F32, name="psb", tag="psb")
                    npass = len(passes)
                    for ki, kt in enumerate(kts):
                        for pi, (lhs_t, rhs_t) in enumerate(passes):
                            for gi in range(GI):
                                i = g0 + gi
                                nc.tensor.matmul(
                                    ps[:, gi * m:(gi + 1) * m],
                                    lhs_t[kt][:, i * m + mt * 128: i * m + mt * 128 + 128],
                                    rhs_t[kt][:, i * m:(i + 1) * m],
                                    start=(pi == 0 and ki == 0 and gi % 2 == 0),
                                    stop=(pi == npass - 1 and ki == len(kts) - 1
                                          and gi % 2 == 1),
                                )
                    post(mt, g0, GI, ps)

        def s_build(last):
            """S = AAt - A_sig A_sig^T; Sh (+Sl if last). Mask lhs once per
            (kt,item); both mt psums live."""
            GI = 8
            for g0 in range(0, I, GI):
                pss = [ps_p.tile([128, 2048], F32, name="psb", tag="psb") for _ in range(MT)]
                for ki, kt in enumerate(range(KT)):
                    for gi in range(GI):
                        i = g0 + gi
                        mk_hi = msk_p.tile([128, m], BF16, name="mskh", tag="mskh")
                        nc.vector.tensor_scalar(
                            mk_hi[:], AT[kt][:, i * m:(i + 1) * m],
                            sig[:, kt * I + i:kt * I + i + 1], None, AluOpType.mult)
                        for mt in range(MT):
                            nc.tensor.matmul(
                                pss[mt][:, gi * m:(gi + 1) * m],
                                mk_hi[:, mt * 128:mt * 128 + 128],
                                AT[kt][:, i * m:(i + 1) * m],
                                start=(ki == 0 and gi % 2 == 0),
                                stop=(ki == KT - 1 and gi % 2 == 1))
                for mt in range(MT):
                    sl_c = slice(g0 * m, (g0 + GI) * m)
                    tmp = scr_p.tile([128, 2048], F32, name="chunk", tag="chunk")
                    nc.vector.tensor_copy(tmp[:], AAtl[mt][:, sl_c])
                    nc.vector.tensor_tensor(tmp[:], tmp[:], pss[mt][:], AluOpType.subtract)
                    nc.vector.tensor_tensor(tmp[:], AAth[mt][:, sl_c], tmp[:], AluOpType.add)
                    nc.vector.tensor_copy(Sh[mt][:, sl_c], tmp[:])
                    if last:
                        nc.vector.tensor_tensor(tmp[:], tmp[:], Sh[mt][:, sl_c],
                                                AluOpType.subtract)
                        nc.vector.tensor_copy(Sl[mt][:, sl_c], tmp[:])

        a_rows = a_d.rearrange("(i p) n -> i p n", p=m)  # [I, m, n]

        def l1hi_stream(ki, c0, CH):
            t = str_p.tile([128, 2048], BF16, name="l1c", tag="l1c", bufs=2)
            src = a_rows[c0:c0 + CH, ki * 128:(ki + 1) * 128, :].rearrange(
                "i p n -> p i n")
            nc.gpsimd.dma_start(out=t[:].rearrange("p (i n) -> p i n", n=n),
                                in_=src)
            return t[:]

        def msp_mv(col_out, vi, Qh, Ql, split):
            """col_out[:, o*I+i] = sum_k Q[o,k](i) @ v[k](i); vi interleaved
            (hi,lo) bf16 pairs; Q symmetric, SBUF-resident."""
            pt = ps_p.tile([128, 2048], F32, name="psb", tag="psb")
            ops = [Qh, Ql] if split else [Qh]
            for o in range(MT):
                for i in range(I):
                    c2 = 2 * (o * I + i)
                    for pi, Q in enumerate(ops):
                        for k in range(MT):
                            nc.tensor.matmul(
                                pt[:, c2:c2 + 2],
                                Q[k][:, i * m + o * 128: i * m + o * 128 + 128],
                                vi[:, 2 * (k * I + i): 2 * (k * I + i) + 2],
                                start=(pi == 0 and k == 0),
                                stop=(pi == len(ops) - 1 and k == MT - 1))
            pe = pt[:, 0:2 * MT * I].rearrange("p (c t) -> p c t", t=2)
            nc.vector.tensor_copy(mtmp[:], pe[:, :, 0])
            nc.vector.tensor_tensor(col_out[:], mtmp[:], pe[:, :, 1],
                                    AluOpType.add)

        def prec_mv(col_out, vh):
            pt = ps_p.tile([128, 2048], F32, name="psb", tag="psb")
            for o in range(MT):
                for i in range(I):
                    c = o * I + i
                    for k in range(MT):
                        nc.tensor.matmul(
                            pt[:, c:c + 1],
                            Mh[k][:, i * m + o * 128: i * m + o * 128 + 128],
                            vh[:, k * I + i: k * I + i + 1],
                            start=(k == 0), stop=(k == MT - 1))
            nc.vector.tensor_copy(col_out[:], pt[:, 0:MT * I])

        def dn_mv(col_out, ui):
            """A @ v (n -> m), ui interleaved n-space pairs."""
            pt = ps_p.tile([128, 2048], F32, name="psb", tag="psb")
            for o in range(MT):
                for i in range(I):
                    c2 = 2 * (o * I + i)
                    for kt in range(KT):
                        nc.tensor.matmul(
                            pt[:, c2:c2 + 2],
                            AT[kt][:, i * m + o * 128: i * m + o * 128 + 128],
                            ui[:, 2 * (kt * I + i): 2 * (kt * I + i) + 2],
                            start=(kt == 0), stop=(kt == KT - 1))
            pe = pt[:, 0:2 * MT * I].rearrange("p (c t) -> p c t", t=2)
            nc.vector.tensor_copy(mtmp[:], pe[:, :, 0])
            nc.vector.tensor_tensor(col_out[:], mtmp[:], pe[:, :, 1],
                                    AluOpType.add)

        def up_mv(col_out, gi_t):
            """A^T @ w (m -> n), gi_t interleaved m-space pairs; A row-layout
            blocks streamed from DRAM as stationaries."""
            ov = col_out.rearrange("p (t i) -> p t i", i=I)
            for c0 in range(0, I, 2):
                tiles = [l1hi_stream(mt, c0, 2) for mt in range(MT)]
                pt = ps_p.tile([128, 2048], F32, name="psb", tag="psb")
                for i_rel in range(2):
                    i = c0 + i_rel
                    for kt in range(KT):
                        c2 = 2 * (i_rel * KT + kt)
                        for mt in range(MT):
                            nc.tensor.matmul(
                                pt[:, c2:c2 + 2],
                                tiles[mt][:, i_rel * n + kt * 128:
                                          i_rel * n + (kt + 1) * 128],
                                gi_t[:, 2 * (mt * I + i): 2 * (mt * I + i) + 2],
                                start=(mt == 0), stop=(mt == MT - 1))
                pe = pt[:, 0:4 * KT].rearrange("p (i t w) -> p t i w", i=2, w=2)
                me = mtmp[:, 0:2 * KT].rearrange("p (t i) -> p t i", i=2)
                nc.vector.tensor_copy(me, pe[:, :, :, 0])
                nc.vector.tensor_tensor(ov[:, :, c0:c0 + 2],
                                        me, pe[:, :, :, 1],
                                        AluOpType.add)

        def cheb(Qh, Ql, iters, l, u, use_split, warm):
            for k, (beta, gamma) in enumerate(_cheb_coeffs(l, u, iters)):
                if k == 0 and not warm:
                    # w=0: residual is exactly rhs; skip the operator apply
                    nc.vector.tensor_copy(gbh[:], rhsc[:])
                    prec_mv(mtmp, gbh)
                    nc.vector.tensor_scalar(wcol[:], mtmp[:], gamma, None, AluOpType.mult)
                    nc.vector.tensor_copy(wprev[:], wcol[:])
                    continue
                split_i(gbi, wcol, mtmp)
                msp_mv(rcol, gbi, Qh, Ql, use_split)
                nc.vector.tensor_tensor(rcol[:], rhsc[:], rcol[:], AluOpType.subtract)
                nc.vector.tensor_copy(gbh[:], rcol[:])
                prec_mv(mtmp, gbh)
                nc.vector.tensor_tensor(wtmp[:], wcol[:], wprev[:], AluOpType.subtract)
                nc.vector.tensor_copy(wprev[:], wcol[:])
                nc.vector.scalar_tensor_tensor(wtmp[:], wtmp[:], beta, wcol[:],
                                               AluOpType.mult, AluOpType.add)
                nc.vector.scalar_tensor_tensor(wcol[:], mtmp[:], gamma, wtmp[:],
                                               AluOpType.mult, AluOpType.add)

        # ============ loads ============
        nc.sync.dma_start(out=xzv[:], in_=xz_d[:])
        nc.sync.dma_start(out=bcol[:], in_=bc_d[:])
        nc.sync.dma_start(out=m01v[:], in_=m01_d[:])
        make_identity(nc, idt[:])
        # IDL[mt][p, i*m + mt*128 + p] = 1 (identity blocks, all items)
        for mt in range(MT):
            nc.gpsimd.memset(IDL[mt][:], 0.0)
            for i in range(I):
                nc.gpsimd.affine_select(
                    out=IDL[mt][:, i * m:(i + 1) * m],
                    in_=IDL[mt][:, i * m:(i + 1) * m],
                    compare_op=AluOpType.not_equal,
                    fill=1.0,
                    base=mt * 128,
                    pattern=[[-1, m]],
                    channel_multiplier=1,
                )
        # A native rows -> AT (A^T layout) via PE transposes
        for mt in range(MT):
            for i0 in range(0, I, 2):
                ps_t = ps_p.tile([128, 2048], BF16, name="psb", tag="psb")
                for di in range(2):
                    i = i0 + di
                    ac = str_p.tile([128, 2048], BF16, name="l1c", tag="l1c", bufs=2)
                    nc.gpsimd.dma_start(out=ac[:, 0:n], in_=a_rows[i, mt * 128:(mt + 1) * 128, :])
                    for kt in range(KT):
                        nc.tensor.transpose(
                            ps_t[:, di * 1024 + kt * 128: di * 1024 + (kt + 1) * 128],
                            ac[:, kt * 128:(kt + 1) * 128], idt[:])
                for di in range(2):
                    i = i0 + di
                    for kt in range(KT):
                        nc.vector.tensor_copy(
                            AT[kt][:, i * m + mt * 128: i * m + mt * 128 + 128],
                            ps_t[:, di * 1024 + kt * 128: di * 1024 + (kt + 1) * 128])

        # ============ AAt = A A^T + eps I ============
        def post_aat(mt, g0, GI, ps):
            sl_c = slice(g0 * m, (g0 + GI) * m)
            tmp = scr_p.tile([128, 2048], F32, name="chunk", tag="chunk")
            nc.vector.scalar_tensor_tensor(tmp[:], IDL[mt][:, sl_c], EPS, ps[:],
                                           AluOpType.mult, AluOpType.add)
            nc.vector.tensor_copy(AAth[mt][:, sl_c], tmp[:])
            nc.vector.tensor_tensor(tmp[:], tmp[:], AAth[mt][:, sl_c], AluOpType.subtract)
            nc.vector.tensor_copy(AAtl[mt][:, sl_c], tmp[:])
        mm_batch([(AT, AT)], range(KT), post_aat)

        # ============ Mt: Newton-Schulz bf16 ============
        assert NS_ITERS % 2 == 1
        Xbufs = [Sl, Mh]   # ping-pong; X0 -> Sl, final (odd) lands in Mh
        for mt in range(MT):
            for c0 in range(0, IM, 2048):
                tmp = scr_p.tile([128, 2048], F32, name="chunk", tag="chunk")
                nc.vector.tensor_scalar(tmp[:], AAth[mt][:, c0:c0 + 2048], -BIN, None,
                                        AluOpType.mult)
                nc.vector.scalar_tensor_tensor(tmp[:], IDL[mt][:, c0:c0 + 2048], AIN,
                                               tmp[:], AluOpType.mult, AluOpType.add)
                nc.vector.tensor_copy(Xbufs[0][mt][:, c0:c0 + 2048], tmp[:])
        for it in range(NS_ITERS):
            Xcur = Xbufs[it % 2]
            Xnxt = Xbufs[(it + 1) % 2]
            def post_p1(mt, g0, GI, ps):
                nc.vector.tensor_copy(Hb[mt][:, g0 * m:(g0 + GI) * m], ps[:])
            mm_batch([(AAth, Xcur)], range(MT), post_p1)
            def post_p2(mt, g0, GI, ps, Xc=Xcur, Xn=Xnxt):
                sl_c = slice(g0 * m, (g0 + GI) * m)
                nc.vector.scalar_tensor_tensor(Xn[mt][:, sl_c], Xc[mt][:, sl_c], 2.0,
                                               ps[:], AluOpType.mult, AluOpType.subtract)
            mm_batch([(Xcur, Hb)], range(MT), post_p2)

        # ============ z0, t2 ============
        split_i(ubi, xzv, ztmp)
        dn_mv(gcol, ubi)
        nc.vector.tensor_tensor(gcol[:], gcol[:], bcol[:], AluOpType.subtract)
        nc.vector.tensor_copy(rhsc[:], gcol[:])
        cheb(AAth, AAtl, BND, 0.80, 1.25, True, warm=False)
        nc.vector.tensor_copy(hcol[:], wcol[:])
        # t2 = b + eps*(AAt^-1 b): the eps term is ~1e-6*|b|, below the bf16-A
        # error floor by 4 orders; use t2 = b directly.
        nc.vector.tensor_copy(t2col[:], bcol[:])
        split_i(gbi, hcol, mtmp)
        up_mv(z0v, gbi)
        nc.vector.tensor_tensor(z0v[:], xzv[:], z0v[:], AluOpType.subtract)

        # ============ rounds ============
        nc.vector.tensor_copy(zv[:], z0v[:])
        Sh = IDL  # identity dead from here; tags sh0/sh1 reused as Sh
        for r in range(N_ROUNDS):
            last = r == N_ROUNDS - 1
            nc.vector.tensor_scalar(sig[:], zv[:], 0.0, None, AluOpType.is_lt)
            nc.vector.tensor_tensor(sig[:], sig[:], m01v[:], AluOpType.mult)
            s_build(last)
            nc.vector.scalar_tensor_tensor(uv[:], sig[:], 0.0, z0v[:],
                                           AluOpType.is_equal, AluOpType.mult)
            split_i(ubi, uv, ztmp)
            dn_mv(rhsc, ubi)
            nc.vector.tensor_tensor(rhsc[:], t2col[:], rhsc[:], AluOpType.subtract)
            cheb(Sh, Sl, RICH[r], 0.07, 1.30, use_split=last, warm=(r > 0))
            split_i(gbi, wcol, mtmp)
            up_mv(zv, gbi)
            nc.vector.tensor_tensor(zv[:], z0v[:], zv[:], AluOpType.add)

        # ============ final ============
        nc.vector.tensor_scalar(sig[:], zv[:], 0.0, None, AluOpType.is_lt)
        nc.vector.tensor_tensor(sig[:], sig[:], m01v[:], AluOpType.mult)
        nc.vector.scalar_tensor_tensor(uv[:], sig[:], 0.0, zv[:],
                                       AluOpType.is_equal, AluOpType.mult)
        split_i(ubi, uv, ztmp)
        dn_mv(gcol, ubi)
        nc.vector.tensor_tensor(gcol[:], gcol[:], bcol[:], AluOpType.subtract)
        nc.vector.tensor_copy(rhsc[:], gcol[:])
        cheb(AAth, AAtl, BND, 0.80, 1.25, True, warm=False)
        split_i(gbi, wcol, mtmp)
        up_mv(ztmp, gbi)
        nc.vector.tensor_tensor(ztmp[:], uv[:], ztmp[:], AluOpType.subtract)
        for i in range(I):
            src = ztmp.rearrange("p (t i) -> p t i", i=I)[:, :, i]
            dst = out_d[i, :].rearrange("(t p) -> p t", p=128)
            nc.sync.dma_start(out=dst, in_=src)

    nc.compile()
    return nc


_SHIMMED = False


def _fix_cc_flags():
    """Route static DMAs through SP so multi-wait DMAs are legal walrus
    codegen (the embedded-wait form only fits one sync wait)."""
    global _SHIMMED
    try:
        from concourse.compiler_utils import get_compiler_flags, set_compiler_flags
        flags = get_compiler_flags()
        nf = [f.replace("--assign-static-dmas-to-sp=false",
                        "--assign-static-dmas-to-sp=true") for f in flags]
        if nf != flags:
            set_compiler_flags(nf)
    except Exception:
        pass
    if not _SHIMMED:
        import concourse.bass_utils as BU
        orig = BU.run_command

        def patched(cmd, *a, **k):
            if isinstance(cmd, (list, tuple)):
                cmd = [str(c).replace("--assign-static-dmas-to-sp=false",
                                      "--assign-static-dmas-to-sp=true") for c in cmd]
            return orig(cmd, *a, **k)

        BU.run_command = patched
        _SHIMMED = True


def _get_runner():
    """Build (once) the Bass module and a cached jitted shard_map callable.
    Returns run(arrays: list[np.ndarray]) -> np.ndarray [B, n]."""
    if "runner" in _CACHE:
        return _CACHE["runner"]
    _fix_cc_flags()
    import jax
    from jax.sharding import Mesh, PartitionSpec
    from jax.experimental.shard_map import shard_map
    from concourse.bass2jax import _bass_exec_p, partition_id_tensor, install_neuronx_cc_hook

    install_neuronx_cc_hook()
    nc = _build()

    partition_name = nc.partition_id_tensor.name if nc.partition_id_tensor else None
    in_names, out_names, out_avals, zero_shapes = [], [], [], []
    for alloc in nc.m.functions[0].allocations:
        if not isinstance(alloc, mybir.MemoryLocationSet):
            continue
        name = alloc.memorylocations[0].name
        if alloc.kind == "ExternalInput":
            if name != partition_name:
                in_names.append(name)
        elif alloc.kind == "ExternalOutput":
            shape = tuple(alloc.tensor_shape)
            dtype = mybir.dt.np(alloc.dtype)
            out_names.append(name)
            out_avals.append(jax.core.ShapedArray(shape, dtype))
            zero_shapes.append((shape, dtype))
    n_params = len(in_names)
    n_outs = len(out_avals)
    in_names_all = in_names + out_names + ([partition_name] if partition_name else [])

    def _body(*args):
        operands = list(args)
        if partition_name is not None:
            operands.append(partition_id_tensor())
        outs = _bass_exec_p.bind(
            *operands,
            out_avals=tuple(out_avals),
            in_names=tuple(in_names_all),
            out_names=tuple(out_names),
            lowering_input_output_aliases=(),
            sim_require_finite=True,
            sim_require_nnan=True,
            nc=nc,
        )
        return tuple(outs)

    devices = jax.devices()[:NCORES]
    mesh = Mesh(np.asarray(devices), ("core",))
    in_specs = (PartitionSpec("core"),) * (n_params + n_outs)
    out_specs = (PartitionSpec("core"),) * n_outs
    donate = tuple(range(n_params, n_params + n_outs))
    sharded = jax.jit(
        shard_map(_body, mesh=mesh, in_specs=in_specs, out_specs=out_specs,
                  check_rep=False),
        donate_argnums=donate, keep_unused=True)

    def run(arr_map):
        args = [arr_map[nm] for nm in in_names]
        zeros = [np.zeros((NCORES * s[0], *s[1:]), dt) for s, dt in zero_shapes]
        outs = sharded(*args, *zeros)
        out = np.asarray(outs[out_names.index("out")])
        return out.reshape(B, n)

    _CACHE["parts"] = dict(sharded=sharded, in_names=in_names,
                           out_names=out_names, zero_shapes=zero_shapes,
                           mesh=mesh, nc=nc)
    _CACHE["runner"] = run
    return run


def _prep_globals(x, b, A, mask):
    a_g = np.ascontiguousarray(A.reshape(B * m, n)).astype(ml_dtypes.bfloat16)
    xz_g = np.ascontiguousarray(
        x.reshape(NCORES, I, KT, 128).transpose(0, 3, 2, 1)).reshape(NCORES * 128, KT * I)
    bc_g = np.ascontiguousarray(
        b.reshape(NCORES, I, MT, 128).transpose(0, 3, 2, 1)).reshape(NCORES * 128, MT * I)
    m01_1 = np.ascontiguousarray(
        np.broadcast_to(mask.reshape(KT, 128, 1), (KT, 128, I)).transpose(1, 0, 2)
    ).reshape(128, KT * I).astype(np.float32)
    m01_g = np.ascontiguousarray(np.tile(m01_1, (NCORES, 1)))
    return {"a": a_g, "xz": xz_g.astype(np.float32), "bc": bc_g.astype(np.float32),
            "m01": m01_g}


def kernel(x, b, A, nonnegative_mask):
    x = np.asarray(x, dtype=np.float32)
    b = np.asarray(b, dtype=np.float32)
    A = np.asarray(A, dtype=np.float32)
    mk = np.asarray(nonnegative_mask).astype(np.float32)
    run = _get_runner()
    arr_map = _prep_globals(x, b, A, mk)
    return np.ascontiguousarray(run(arr_map)).astype(np.float32)


# revision 19
# speedup vs baseline: 1.5811x; 1.1160x over previous
"""Trainium2 Bass kernel: batched QP projection (Dykstra fixed point) via an
active-set direct solve. Data parallel: 8 NeuronCores x 16 items each.

Per item (validated offline vs reference, rel err ~2.0e-3, gate 2e-2; the
error floor is bf16 quantization of A, not solver convergence):
  AAt = A A^T + eps I;  Mt ~= inv(AAt) (bf16 Newton-Schulz; preconditioner)
  z0 = x - A^T h0 with AAt h0 = (A x - b)   (preconditioned Chebyshev)
  3 rounds: sigma = (z<0)&mask
     S = AAt - A_sig A_sig^T  ( = A D A^T + eps I, D = diag(1-sigma) )
     solve S w = b - A (D z0)  by Chebyshev (Mt-preconditioned), warm start
     z = z0 + A^T w
  out = x* - A^T h with AAt h = (A x* - b), x* = (1-sigma) z

A ships ONCE, bf16, in native [I*m, n] layout (one host cast, no transpose
passes, no mask permute); the device builds the A^T layout via PE-array
transposes and streams the native layout for A^T-matvecs. AAt/S are split
hi/lo bf16 on device for fp32-quality Chebyshev operators. All matvecs are
thin per-item matmuls (matrix stationary, interleaved hi/lo vector pairs as
2-wide rhs) writing results directly in column layout to PSUM - no DRAM
bounce, and the vector hi/lo split costs nothing extra. The jitted PJRT
callable is cached so warm calls skip retrace/recompile.
"""

import sys

for _p in ("/opt/trn_rl_repo", "/opt/pypackages"):
    if _p not in sys.path:
        sys.path.insert(0, _p)

import numpy as np
import ml_dtypes
from contextlib import ExitStack

import concourse.bass as bass
import concourse.tile as tile
from concourse import mybir, bacc
from concourse.alu_op_type import AluOpType
from concourse.masks import make_identity

F32 = mybir.dt.float32
BF16 = mybir.dt.bfloat16

B, m, n = 128, 256, 1024
NCORES = 8
I = B // NCORES      # 16
KT = n // 128        # 8
MT = m // 128        # 2
IM = I * m           # 4096
IN = I * n           # 16384
EPS = 1e-6

N_ROUNDS = 3
RICH = [5, 4, 4]
NS_ITERS = 1
BND = 3            # boundary cheb iterations (z0/final solves)
AIN, BIN = 0.8340, 0.2173

_CACHE = {}


def _cheb_coeffs(l, u, iters):
    th, dl = (u + l) / 2.0, (u - l) / 2.0
    sg = th / dl
    out = []
    rho_prev = None
    for k in range(iters):
        if k == 0:
            out.append((0.0, 1.0 / th))
            rho_prev = 1.0 / sg
        else:
            rho = 1.0 / (2.0 * sg - rho_prev)
            out.append((rho * rho_prev, 2.0 * rho / dl))
            rho_prev = rho
    return out  # (beta_k, gamma_k): w_new = w + beta*(w - wprev) + gamma*z


def _build():
    nc = bacc.Bacc("TRN2", target_bir_lowering=False, debug=False, num_devices=NCORES)
    a_d = nc.declare_dram_parameter("a", [I * m, n], BF16, isOutput=False)
    xz_d = nc.declare_dram_parameter("xz", [128, KT * I], F32, isOutput=False)
    bc_d = nc.declare_dram_parameter("bc", [128, MT * I], F32, isOutput=False)
    m01_d = nc.declare_dram_parameter("m01", [128, KT * I], F32, isOutput=False)
    out_d = nc.declare_dram_parameter("out", [I, n], F32, isOutput=True)

    with tile.TileContext(nc) as tc, ExitStack() as ctx:
        nc = tc.nc
        ath_p = ctx.enter_context(tc.tile_pool(name="ath", bufs=1))
        res_p = ctx.enter_context(tc.tile_pool(name="res", bufs=1))
        scr_p = ctx.enter_context(tc.tile_pool(name="scr", bufs=2))
        msk_p = ctx.enter_context(tc.tile_pool(name="msk", bufs=4))
        str_p = ctx.enter_context(tc.tile_pool(name="str", bufs=3))
        vec_p = ctx.enter_context(tc.tile_pool(name="vec", bufs=1))
        ps_p = ctx.enter_context(tc.tile_pool(name="ps", bufs=2, space=bass.MemorySpace.PSUM))

        AT = [ath_p.tile([128, IM], BF16, name=f"ath{k}", tag=f"ath{k}") for k in range(KT)]
        AAth = [res_p.tile([128, IM], BF16, name=f"aah{k}", tag=f"aah{k}") for k in range(MT)]
        AAtl = [res_p.tile([128, IM], BF16, name=f"aal{k}", tag=f"aal{k}") for k in range(MT)]
        Mh = [res_p.tile([128, IM], BF16, name=f"mh{k}", tag=f"mh{k}") for k in range(MT)]
        IDL = [res_p.tile([128, IM], BF16, name=f"sh{k}", tag=f"sh{k}") for k in range(MT)]  # -> Sh later
        Sl = [res_p.tile([128, IM], BF16, name=f"sl{k}", tag=f"sl{k}") for k in range(MT)]
        Hb = [str_p.tile([128, IM], BF16, name=f"hbc{k}", tag="hbc", bufs=2) for k in range(MT)]  # NS-only

        zv = vec_p.tile([128, KT * I], F32, name="zv", tag="zv")
        z0v = vec_p.tile([128, KT * I], F32, name="z0v", tag="z0v")
        uv = vec_p.tile([128, KT * I], F32, name="uv", tag="uv")
        sig = vec_p.tile([128, KT * I], F32, name="sig", tag="sig")
        m01v = vec_p.tile([128, KT * I], F32, name="m01v", tag="m01v")
        xzv = vec_p.tile([128, KT * I], F32, name="xzv", tag="xzv")
        ubi = vec_p.tile([128, 2 * KT * I], BF16, name="ubi", tag="ubi")
        ztmp = vec_p.tile([128, KT * I], F32, name="ztmp", tag="ztmp")
        bcol = vec_p.tile([128, MT * I], F32, name="bcol", tag="bcol")
        gcol = vec_p.tile([128, MT * I], F32, name="gcol", tag="gcol")
        hcol = vec_p.tile([128, MT * I], F32, name="hcol", tag="hcol")
        wcol = vec_p.tile([128, MT * I], F32, name="wcol", tag="wcol")
        wprev = vec_p.tile([128, MT * I], F32, name="wprev", tag="wprev")
        wtmp = vec_p.tile([128, MT * I], F32, name="wtmp", tag="wtmp")
        t2col = vec_p.tile([128, MT * I], F32, name="t2col", tag="t2col")
        rhsc = vec_p.tile([128, MT * I], F32, name="rhsc", tag="rhsc")
        rcol = vec_p.tile([128, MT * I], F32, name="rcol", tag="rcol")
        mtmp = vec_p.tile([128, MT * I], F32, name="mtmp", tag="mtmp")
        gbh = vec_p.tile([128, MT * I], BF16, name="gbh", tag="gbh")
        gbi = vec_p.tile([128, 2 * MT * I], BF16, name="gbi", tag="gbi")
        idt = vec_p.tile([128, 128], BF16, name="idt", tag="idt")

        # ---------------- helpers ----------------
        def split_i(dst, src, tmp):
            """dst bf16 [128, 2*C] interleaved (hi,lo) pairs from f32 src."""
            d3 = dst[:].rearrange("p (c t) -> p c t", t=2)
            nc.vector.tensor_copy(d3[:, :, 0], src[:])
            nc.vector.tensor_tensor(tmp[:], src[:], d3[:, :, 0], AluOpType.subtract)
            nc.vector.tensor_copy(d3[:, :, 1], tmp[:])

        def mm_batch(passes, kts, post):
            """out[i] = sum_passes lhsT[i].T @ rhs[i] over kts; psum chunks of
            8 items; post(mt, g0, GI, ps)."""
            GI = 8
            for mt in range(MT):
                for g0 in range(0, I, GI):
                    ps = ps_p.tile([128, 2048], F32, name="psb", tag="psb")
                    npass = len(passes)
                    for ki, kt in enumerate(kts):
                        for pi, (lhs_t, rhs_t) in enumerate(passes):
                            for gi in range(GI):
                                i = g0 + gi
                                nc.tensor.matmul(
                                    ps[:, gi * m:(gi + 1) * m],
                                    lhs_t[kt][:, i * m + mt * 128: i * m + mt * 128 + 128],
                                    rhs_t[kt][:, i * m:(i + 1) * m],
                                    start=(pi == 0 and ki == 0 and gi % 2 == 0),
                                    stop=(pi == npass - 1 and ki == len(kts) - 1
                                          and gi % 2 == 1),
                                )
                    post(mt, g0, GI, ps)

        def s_build(last):
            """S = AAt - A_sig A_sig^T; Sh (+Sl if last). Mask lhs once per
            (kt,item); both mt psums live."""
            GI = 8
            for g0 in range(0, I, GI):
                pss = [ps_p.tile([128, 2048], F32, name="psb", tag="psb") for _ in range(MT)]
                for ki, kt in enumerate(range(KT)):
                    for gi in range(GI):
                        i = g0 + gi
                        mk_hi = msk_p.tile([128, m], BF16, name="mskh", tag="mskh")
                        nc.vector.tensor_scalar(
                            mk_hi[:], AT[kt][:, i * m:(i + 1) * m],
                            sig[:, kt * I + i:kt * I + i + 1], None, AluOpType.mult)
                        for mt in range(MT):
                            nc.tensor.matmul(
                                pss[mt][:, gi * m:(gi + 1) * m],
                                mk_hi[:, mt * 128:mt * 128 + 128],
                                AT[kt][:, i * m:(i + 1) * m],
                                start=(ki == 0 and gi % 2 == 0),
                                stop=(ki == KT - 1 and gi % 2 == 1))
                for mt in range(MT):
                    sl_c = slice(g0 * m, (g0 + GI) * m)
                    tmp = scr_p.tile([128, 2048], F32, name="chunk", tag="chunk")
                    nc.vector.tensor_copy(tmp[:], AAtl[mt][:, sl_c])
                    nc.vector.tensor_tensor(tmp[:], tmp[:], pss[mt][:], AluOpType.subtract)
                    nc.vector.tensor_tensor(tmp[:], AAth[mt][:, sl_c], tmp[:], AluOpType.add)
                    nc.vector.tensor_copy(Sh[mt][:, sl_c], tmp[:])
                    if last:
                        nc.vector.tensor_tensor(tmp[:], tmp[:], Sh[mt][:, sl_c],
                                                AluOpType.subtract)
                        nc.vector.tensor_copy(Sl[mt][:, sl_c], tmp[:])

        a_rows = a_d.rearrange("(i p) n -> i p n", p=m)  # [I, m, n]

        def l1hi_stream(ki, c0, CH):
            t = str_p.tile([128, 2048], BF16, name="l1c", tag="l1c", bufs=2)
            src = a_rows[c0:c0 + CH, ki * 128:(ki + 1) * 128, :].rearrange(
                "i p n -> p i n")
            nc.gpsimd.dma_start(out=t[:].rearrange("p (i n) -> p i n", n=n),
                                in_=src)
            return t[:]

        def msp_mv(col_out, vi, Qh, Ql, split):
            """col_out[:, o*I+i] = sum_k Q[o,k](i) @ v[k](i); vi interleaved
            (hi,lo) bf16 pairs; Q symmetric, SBUF-resident."""
            pt = ps_p.tile([128, 2048], F32, name="psb", tag="psb")
            ops = [Qh, Ql] if split else [Qh]
            for o in range(MT):
                for i in range(I):
                    c2 = 2 * (o * I + i)
                    for pi, Q in enumerate(ops):
                        for k in range(MT):
                            nc.tensor.matmul(
                                pt[:, c2:c2 + 2],
                                Q[k][:, i * m + o * 128: i * m + o * 128 + 128],
                                vi[:, 2 * (k * I + i): 2 * (k * I + i) + 2],
                                start=(pi == 0 and k == 0),
                                stop=(pi == len(ops) - 1 and k == MT - 1))
            pe = pt[:, 0:2 * MT * I].rearrange("p (c t) -> p c t", t=2)
            nc.vector.tensor_copy(mtmp[:], pe[:, :, 0])
            nc.vector.tensor_tensor(col_out[:], mtmp[:], pe[:, :, 1],
                                    AluOpType.add)

        def prec_mv(col_out, vh):
            pt = ps_p.tile([128, 2048], F32, name="psb", tag="psb")
            for o in range(MT):
                for i in range(I):
                    c = o * I + i
                    for k in range(MT):
                        nc.tensor.matmul(
                            pt[:, c:c + 1],
                            Mh[k][:, i * m + o * 128: i * m + o * 128 + 128],
                            vh[:, k * I + i: k * I + i + 1],
                            start=(k == 0), stop=(k == MT - 1))
            nc.vector.tensor_copy(col_out[:], pt[:, 0:MT * I])

        def dn_mv(col_out, ui):
            """A @ v (n -> m), ui interleaved n-space pairs."""
            pt = ps_p.tile([128, 2048], F32, name="psb", tag="psb")
            for o in range(MT):
                for i in range(I):
                    c2 = 2 * (o * I + i)
                    for kt in range(KT):
                        nc.tensor.matmul(
                            pt[:, c2:c2 + 2],
                            AT[kt][:, i * m + o * 128: i * m + o * 128 + 128],
                            ui[:, 2 * (kt * I + i): 2 * (kt * I + i) + 2],
                            start=(kt == 0), stop=(kt == KT - 1))
            pe = pt[:, 0:2 * MT * I].rearrange("p (c t) -> p c t", t=2)
            nc.vector.tensor_copy(mtmp[:], pe[:, :, 0])
            nc.vector.tensor_tensor(col_out[:], mtmp[:], pe[:, :, 1],
                                    AluOpType.add)

        def up_mv(col_out, gi_t):
            """A^T @ w (m -> n), gi_t interleaved m-space pairs; A row-layout
            blocks streamed from DRAM as stationaries."""
            ov = col_out.rearrange("p (t i) -> p t i", i=I)
            for c0 in range(0, I, 2):
                tiles = [l1hi_stream(mt, c0, 2) for mt in range(MT)]
                pt = ps_p.tile([128, 2048], F32, name="psb", tag="psb")
                for i_rel in range(2):
                    i = c0 + i_rel
                    for kt in range(KT):
                        c2 = 2 * (i_rel * KT + kt)
                        for mt in range(MT):
                            nc.tensor.matmul(
                                pt[:, c2:c2 + 2],
                                tiles[mt][:, i_rel * n + kt * 128:
                                          i_rel * n + (kt + 1) * 128],
                                gi_t[:, 2 * (mt * I + i): 2 * (mt * I + i) + 2],
                                start=(mt == 0), stop=(mt == MT - 1))
                pe = pt[:, 0:4 * KT].rearrange("p (i t w) -> p t i w", i=2, w=2)
                me = mtmp[:, 0:2 * KT].rearrange("p (t i) -> p t i", i=2)
                nc.vector.tensor_copy(me, pe[:, :, :, 0])
                nc.vector.tensor_tensor(ov[:, :, c0:c0 + 2],
                                        me, pe[:, :, :, 1],
                                        AluOpType.add)

        def cheb(Qh, Ql, iters, l, u, use_split, warm):
            for k, (beta, gamma) in enumerate(_cheb_coeffs(l, u, iters)):
                if k == 0 and not warm:
                    # w=0: residual is exactly rhs; skip the operator apply
                    nc.vector.tensor_copy(gbh[:], rhsc[:])
                    prec_mv(mtmp, gbh)
                    nc.vector.tensor_scalar(wcol[:], mtmp[:], gamma, None, AluOpType.mult)
                    nc.vector.tensor_copy(wprev[:], wcol[:])
                    continue
                split_i(gbi, wcol, mtmp)
                msp_mv(rcol, gbi, Qh, Ql, use_split)
                nc.vector.tensor_tensor(rcol[:], rhsc[:], rcol[:], AluOpType.subtract)
                nc.vector.tensor_copy(gbh[:], rcol[:])
                prec_mv(mtmp, gbh)
                nc.vector.tensor_tensor(wtmp[:], wcol[:], wprev[:], AluOpType.subtract)
                nc.vector.tensor_copy(wprev[:], wcol[:])
                nc.vector.scalar_tensor_tensor(wtmp[:], wtmp[:], beta, wcol[:],
                                               AluOpType.mult, AluOpType.add)
                nc.vector.scalar_tensor_tensor(wcol[:], mtmp[:], gamma, wtmp[:],
                                               AluOpType.mult, AluOpType.add)

        # ============ loads ============
        nc.sync.dma_start(out=xzv[:], in_=xz_d[:])
        nc.sync.dma_start(out=bcol[:], in_=bc_d[:])
        nc.sync.dma_start(out=m01v[:], in_=m01_d[:])
        make_identity(nc, idt[:])
        # IDL[mt][p, i*m + mt*128 + p] = 1 (identity blocks, all items)
        for mt in range(MT):
            nc.gpsimd.memset(IDL[mt][:], 0.0)
            for i in range(I):
                nc.gpsimd.affine_select(
                    out=IDL[mt][:, i * m:(i + 1) * m],
                    in_=IDL[mt][:, i * m:(i + 1) * m],
                    compare_op=AluOpType.not_equal,
                    fill=1.0,
                    base=mt * 128,
                    pattern=[[-1, m]],
                    channel_multiplier=1,
                )
        # A native rows -> AT (A^T layout) via PE transposes
        for mt in range(MT):
            for i0 in range(0, I, 2):
                ps_t = ps_p.tile([128, 2048], BF16, name="psb", tag="psb")
                for di in range(2):
                    i = i0 + di
                    ac = str_p.tile([128, 2048], BF16, name="l1c", tag="l1c", bufs=2)
                    nc.gpsimd.dma_start(out=ac[:, 0:n], in_=a_rows[i, mt * 128:(mt + 1) * 128, :])
                    for kt in range(KT):
                        nc.tensor.transpose(
                            ps_t[:, di * 1024 + kt * 128: di * 1024 + (kt + 1) * 128],
                            ac[:, kt * 128:(kt + 1) * 128], idt[:])
                for di in range(2):
                    i = i0 + di
                    for kt in range(KT):
                        nc.vector.tensor_copy(
                            AT[kt][:, i * m + mt * 128: i * m + mt * 128 + 128],
                            ps_t[:, di * 1024 + kt * 128: di * 1024 + (kt + 1) * 128])

        # ============ AAt = A A^T + eps I ============
        def post_aat(mt, g0, GI, ps):
            sl_c = slice(g0 * m, (g0 + GI) * m)
            tmp = scr_p.tile([128, 2048], F32, name="chunk", tag="chunk")
            nc.vector.scalar_tensor_tensor(tmp[:], IDL[mt][:, sl_c], EPS, ps[:],
                                           AluOpType.mult, AluOpType.add)
            nc.vector.tensor_copy(AAth[mt][:, sl_c], tmp[:])
            nc.vector.tensor_tensor(tmp[:], tmp[:], AAth[mt][:, sl_c], AluOpType.subtract)
            nc.vector.tensor_copy(AAtl[mt][:, sl_c], tmp[:])
        mm_batch([(AT, AT)], range(KT), post_aat)

        # ============ Mt: Newton-Schulz bf16 ============
        assert NS_ITERS % 2 == 1
        Xbufs = [Sl, Mh]   # ping-pong; X0 -> Sl, final (odd) lands in Mh
        for mt in range(MT):
            for c0 in range(0, IM, 2048):
                tmp = scr_p.tile([128, 2048], F32, name="chunk", tag="chunk")
                nc.vector.tensor_scalar(tmp[:], AAth[mt][:, c0:c0 + 2048], -BIN, None,
                                        AluOpType.mult)
                nc.vector.scalar_tensor_tensor(tmp[:], IDL[mt][:, c0:c0 + 2048], AIN,
                                               tmp[:], AluOpType.mult, AluOpType.add)
                nc.vector.tensor_copy(Xbufs[0][mt][:, c0:c0 + 2048], tmp[:])
        for it in range(NS_ITERS):
            Xcur = Xbufs[it % 2]
            Xnxt = Xbufs[(it + 1) % 2]
            def post_p1(mt, g0, GI, ps):
                nc.vector.tensor_copy(Hb[mt][:, g0 * m:(g0 + GI) * m], ps[:])
            mm_batch([(AAth, Xcur)], range(MT), post_p1)
            def post_p2(mt, g0, GI, ps, Xc=Xcur, Xn=Xnxt):
                sl_c = slice(g0 * m, (g0 + GI) * m)
                nc.vector.scalar_tensor_tensor(Xn[mt][:, sl_c], Xc[mt][:, sl_c], 2.0,
                                               ps[:], AluOpType.mult, AluOpType.subtract)
            mm_batch([(Xcur, Hb)], range(MT), post_p2)

        # ============ z0, t2 ============
        split_i(ubi, xzv, ztmp)
        dn_mv(gcol, ubi)
        nc.vector.tensor_tensor(gcol[:], gcol[:], bcol[:], AluOpType.subtract)
        nc.vector.tensor_copy(rhsc[:], gcol[:])
        cheb(AAth, AAtl, BND, 0.80, 1.25, True, warm=False)
        nc.vector.tensor_copy(hcol[:], wcol[:])
        # t2 = b + eps*(AAt^-1 b): the eps term is ~1e-6*|b|, below the bf16-A
        # error floor by 4 orders; use t2 = b directly.
        nc.vector.tensor_copy(t2col[:], bcol[:])
        split_i(gbi, hcol, mtmp)
        up_mv(z0v, gbi)
        nc.vector.tensor_tensor(z0v[:], xzv[:], z0v[:], AluOpType.subtract)

        # ============ rounds ============
        nc.vector.tensor_copy(zv[:], z0v[:])
        Sh = IDL  # identity dead from here; tags sh0/sh1 reused as Sh
        for r in range(N_ROUNDS):
            last = r == N_ROUNDS - 1
            nc.vector.tensor_scalar(sig[:], zv[:], 0.0, None, AluOpType.is_lt)
            nc.vector.tensor_tensor(sig[:], sig[:], m01v[:], AluOpType.mult)
            s_build(last)
            nc.vector.scalar_tensor_tensor(uv[:], sig[:], 0.0, z0v[:],
                                           AluOpType.is_equal, AluOpType.mult)
            split_i(ubi, uv, ztmp)
            dn_mv(rhsc, ubi)
            nc.vector.tensor_tensor(rhsc[:], t2col[:], rhsc[:], AluOpType.subtract)
            cheb(Sh, Sl, RICH[r], 0.07, 1.30, use_split=last, warm=(r > 0))
            split_i(gbi, wcol, mtmp)
            up_mv(zv, gbi)
            nc.vector.tensor_tensor(zv[:], z0v[:], zv[:], AluOpType.add)

        # ============ final ============
        nc.vector.tensor_scalar(sig[:], zv[:], 0.0, None, AluOpType.is_lt)
        nc.vector.tensor_tensor(sig[:], sig[:], m01v[:], AluOpType.mult)
        nc.vector.scalar_tensor_tensor(uv[:], sig[:], 0.0, zv[:],
                                       AluOpType.is_equal, AluOpType.mult)
        split_i(ubi, uv, ztmp)
        dn_mv(gcol, ubi)
        nc.vector.tensor_tensor(gcol[:], gcol[:], bcol[:], AluOpType.subtract)
        nc.vector.tensor_copy(rhsc[:], gcol[:])
        cheb(AAth, AAtl, BND, 0.80, 1.25, True, warm=False)
        split_i(gbi, wcol, mtmp)
        up_mv(ztmp, gbi)
        nc.vector.tensor_tensor(ztmp[:], uv[:], ztmp[:], AluOpType.subtract)
        for i in range(I):
            src = ztmp.rearrange("p (t i) -> p t i", i=I)[:, :, i]
            dst = out_d[i, :].rearrange("(t p) -> p t", p=128)
            nc.sync.dma_start(out=dst, in_=src)

    nc.compile()
    return nc


_SHIMMED = False


def _fix_cc_flags():
    """Route static DMAs through SP so multi-wait DMAs are legal walrus
    codegen (the embedded-wait form only fits one sync wait)."""
    global _SHIMMED
    try:
        from concourse.compiler_utils import get_compiler_flags, set_compiler_flags
        flags = get_compiler_flags()
        nf = [f.replace("--assign-static-dmas-to-sp=false",
                        "--assign-static-dmas-to-sp=true") for f in flags]
        if nf != flags:
            set_compiler_flags(nf)
    except Exception:
        pass
    if not _SHIMMED:
        import concourse.bass_utils as BU
        orig = BU.run_command

        def patched(cmd, *a, **k):
            if isinstance(cmd, (list, tuple)):
                cmd = [str(c).replace("--assign-static-dmas-to-sp=false",
                                      "--assign-static-dmas-to-sp=true") for c in cmd]
            return orig(cmd, *a, **k)

        BU.run_command = patched
        _SHIMMED = True


def _get_runner():
    """Build (once) the Bass module and a cached jitted shard_map callable.
    Returns run(arrays: list[np.ndarray]) -> np.ndarray [B, n]."""
    if "runner" in _CACHE:
        return _CACHE["runner"]
    _fix_cc_flags()
    import jax
    from jax.sharding import Mesh, PartitionSpec
    from jax.experimental.shard_map import shard_map
    from concourse.bass2jax import _bass_exec_p, partition_id_tensor, install_neuronx_cc_hook

    install_neuronx_cc_hook()
    nc = _build()

    partition_name = nc.partition_id_tensor.name if nc.partition_id_tensor else None
    in_names, out_names, out_avals, zero_shapes = [], [], [], []
    for alloc in nc.m.functions[0].allocations:
        if not isinstance(alloc, mybir.MemoryLocationSet):
            continue
        name = alloc.memorylocations[0].name
        if alloc.kind == "ExternalInput":
            if name != partition_name:
                in_names.append(name)
        elif alloc.kind == "ExternalOutput":
            shape = tuple(alloc.tensor_shape)
            dtype = mybir.dt.np(alloc.dtype)
            out_names.append(name)
            out_avals.append(jax.core.ShapedArray(shape, dtype))
            zero_shapes.append((shape, dtype))
    n_params = len(in_names)
    n_outs = len(out_avals)
    in_names_all = in_names + out_names + ([partition_name] if partition_name else [])

    def _body(*args):
        operands = list(args)
        if partition_name is not None:
            operands.append(partition_id_tensor())
        outs = _bass_exec_p.bind(
            *operands,
            out_avals=tuple(out_avals),
            in_names=tuple(in_names_all),
            out_names=tuple(out_names),
            lowering_input_output_aliases=(),
            sim_require_finite=True,
            sim_require_nnan=True,
            nc=nc,
        )
        return tuple(outs)

    devices = jax.devices()[:NCORES]
    mesh = Mesh(np.asarray(devices), ("core",))
    in_specs = (PartitionSpec("core"),) * (n_params + n_outs)
    out_specs = (PartitionSpec("core"),) * n_outs
    donate = tuple(range(n_params, n_params + n_outs))
    sharded = jax.jit(
        shard_map(_body, mesh=mesh, in_specs=in_specs, out_specs=out_specs,
                  check_rep=False),
        donate_argnums=donate, keep_unused=True)

    def run(arr_map):
        args = [arr_map[nm] for nm in in_names]
        zeros = [np.zeros((NCORES * s[0], *s[1:]), dt) for s, dt in zero_shapes]
        outs = sharded(*args, *zeros)
        out = np.asarray(outs[out_names.index("out")])
        return out.reshape(B, n)

    _CACHE["parts"] = dict(sharded=sharded, in_names=in_names,
                           out_names=out_names, zero_shapes=zero_shapes,
                           mesh=mesh, nc=nc)
    _CACHE["runner"] = run
    return run


def _prep_globals(x, b, A, mask):
    a_g = np.ascontiguousarray(A.reshape(B * m, n)).astype(ml_dtypes.bfloat16)
    xz_g = np.ascontiguousarray(
        x.reshape(NCORES, I, KT, 128).transpose(0, 3, 2, 1)).reshape(NCORES * 128, KT * I)
    bc_g = np.ascontiguousarray(
        b.reshape(NCORES, I, MT, 128).transpose(0, 3, 2, 1)).reshape(NCORES * 128, MT * I)
    m01_1 = np.ascontiguousarray(
        np.broadcast_to(mask.reshape(KT, 128, 1), (KT, 128, I)).transpose(1, 0, 2)
    ).reshape(128, KT * I).astype(np.float32)
    m01_g = np.ascontiguousarray(np.tile(m01_1, (NCORES, 1)))
    return {"a": a_g, "xz": xz_g.astype(np.float32), "bc": bc_g.astype(np.float32),
            "m01": m01_g}


def kernel(x, b, A, nonnegative_mask):
    x = np.asarray(x, dtype=np.float32)
    b = np.asarray(b, dtype=np.float32)
    A = np.asarray(A, dtype=np.float32)
    mk = np.asarray(nonnegative_mask).astype(np.float32)
    run = _get_runner()
    arr_map = _prep_globals(x, b, A, mk)
    return np.ascontiguousarray(run(arr_map)).astype(np.float32)
